# revision 17
# baseline (speedup 1.0000x reference)
"""Trainium2 Bass kernel for nn_BasicConvolutionBlock (sparse 3x3x3 conv + BN + ReLU).

Strategy (8 NeuronCores, data-parallel over the N=500k voxels):
  - Host: make neighbor data local per shard — apply the kernel-map
    (gather + validity mask), quantize to fp8-e3m4 (4 mantissa bits,
    |x| < 15.5 fits; measured end-to-end rel err 1.24e-2 vs the 2e-2
    gate), and lay it out partition-major / tile-contiguous so each
    core streams its shard sequentially at full HBM bandwidth (the
    kernel is HBM-bound: 54 MB/core streamed vs 236 MB for fp32).
  - Device (per core): per 512-voxel tile, 7 accumulating matmuls into
    PSUM (contraction 128 = 4 taps x 32 cin, bf16 weights x fp8
    activations — mixed-dtype PE matmul, verified exact); tile PAIRS
    share one [128,512] PSUM tile via col-group tile_position (even
    tile -> partitions 0:64, odd -> 64:128), which also runs the two
    matmul streams concurrently on the two PE column halves.  BN sum
    rides on the ScalarE PSUM->SBUF fp16 copy (accum_out); sum-of-
    squares on VectorE via tensor_tensor_reduce; cross-core AllReduce
    of (sum, sumsq); then a fused scale/bias/ReLU pass split across
    ScalarE and VectorE.
  - Input DMA chunk sizes are free (tile-contiguous layout): small
    leading chunks start the matmuls early, 8-tile bodies amortize DMA
    overhead.  Output is written channel-major fp16 [128, 62*512]; the
    host undoes the transpose and upcasts.
"""
import sys

sys.path.insert(0, "/opt/trn_rl_repo")

import ml_dtypes
import numpy as np

import concourse.bass as bass
import concourse.bacc as bacc
import concourse.tile as tile
from concourse import mybir, bass_utils

N = 500_000
CIN = 32
COUT = 64
K = 27
EPS = 1e-5
NCORES = 8
NSH = N // NCORES          # 62500 voxels per core
T = 512                    # voxels per tile
NT = 124                   # tiles per core (padded: 124*512 = 63488 >= 62500)
NPAD = NT * T
NPAIR = NT // 2            # 62 tile-pairs
NG = 7                     # tap groups of 4 (27 taps + 1 zero tap)
P2G = 8                    # tile-pairs per phase-2 store group

F32 = mybir.dt.float32
F16 = mybir.dt.float16
BF16 = mybir.dt.bfloat16
FP8 = mybir.dt.float8e3
BF16NP = ml_dtypes.bfloat16
F8NP = ml_dtypes.float8_e3m4

QT = 4                     # tiles per DMA batch ("quad")
NQ = NT // QT              # 31 quads
# quad 0 split in two half-loads so the first matmuls start earlier; the
# last quad likewise so the final PE tail after the last DMA is shorter
CHUNKS = (
    [(0, 0, 2), (0, 2, 2)]
    + [(q, 0, QT) for q in range(1, NQ - 1)]
    + [(NQ - 1, 0, 2), (NQ - 1, 2, 2)]
)


def _build(nc):
    ga_d = nc.dram_tensor("ga", [NQ, 128, QT * 6 * T], FP8, kind="ExternalInput")
    gc_d = nc.dram_tensor("gc", [NQ, 96, QT * T], FP8, kind="ExternalInput")
    w4_d = nc.dram_tensor("w4", [128, NG * COUT], BF16, kind="ExternalInput")
    gbeta_d = nc.dram_tensor("gbeta", [128, 2], F32, kind="ExternalInput")
    y2_d = nc.dram_tensor("y2", [128, NPAIR * T], F16, kind="ExternalOutput")

    with tile.TileContext(nc) as tc:
        with (
            tc.tile_pool(name="persist", bufs=1) as pp,
            tc.tile_pool(name="dram", bufs=1, space="DRAM") as dram,
        ):
            w4_sb = pp.tile([128, NG * COUT], BF16)
            gb_sb = pp.tile([128, 2], F32)   # gamma col0, beta col1 (replicated)
            sums = pp.tile([128, NPAIR], F32)
            sumsq = pp.tile([128, NPAIR], F32)
            out_sb = pp.tile([128, NPAIR * T], F16)
            sb_full = pp.tile([128, 2], F32)  # col0 scale, col1 bias

            nc.sync.dma_start(out=w4_sb[:], in_=w4_d[:, :])
            nc.sync.dma_start(out=gb_sb[:], in_=gbeta_d[:, :])

            # ---- Phase 1: conv matmuls + raw stats ----
            with (
                tc.tile_pool(name="gina", bufs=6) as gina,
                tc.tile_pool(name="ginc", bufs=6) as ginc,
                tc.tile_pool(name="po", bufs=4, space="PSUM") as pop,
                tc.tile_pool(name="sq", bufs=2) as sqp,
            ):
                for q, t0, ntl in CHUNKS:
                    gta = gina.tile([128, ntl * 6 * T], FP8, tag="gta")
                    gtc = ginc.tile([96, ntl * T], FP8, tag="gtc")
                    nc.sync.dma_start(
                        out=gta[:], in_=ga_d[q, :, t0 * 6 * T : (t0 + ntl) * 6 * T]
                    )
                    nc.gpsimd.dma_start(
                        out=gtc[:], in_=gc_d[q, :, t0 * T : (t0 + ntl) * T]
                    )
                    po = None
                    for ti in range(ntl):
                        t = q * QT + t0 + ti
                        pair, half = t // 2, t % 2
                        if half == 0:
                            po = pop.tile([128, T], F32, tag="po")
                        lo = 64 * half
                        for g in range(6):
                            nc.tensor.matmul(
                                out=po[lo : lo + 64, :],
                                lhsT=w4_sb[:, 64 * g : 64 * g + 64],
                                rhs=gta[:, (ti * 6 + g) * T : (ti * 6 + g) * T + T],
                                start=(g == 0),
                                stop=False,
                            )
                        nc.tensor.matmul(
                            out=po[lo : lo + 64, :],
                            lhsT=w4_sb[0:96, 64 * 6 : 64 * 6 + 64],
                            rhs=gtc[:, ti * T : ti * T + T],
                            start=False,
                            stop=True,
                        )
                        if half == 1:
                            nc.scalar.activation(
                                out=out_sb[:, T * pair : T * pair + T],
                                in_=po[:],
                                func=mybir.ActivationFunctionType.Copy,
                                accum_out=sums[:, pair : pair + 1],
                            )
                            sq = sqp.tile([128, T], BF16, tag="sq")
                            nc.scalar.activation(
                                out=sq[:],
                                in_=po[:],
                                func=mybir.ActivationFunctionType.Square,
                                accum_out=sumsq[:, pair : pair + 1],
                            )

            # ---- Stats: reduce, fold halves, all-reduce, scale/bias ----
            s2 = pp.tile([128, 2], F32)
            nc.vector.tensor_reduce(
                out=s2[:, 0:1], in_=sums[:], axis=mybir.AxisListType.X,
                op=mybir.AluOpType.add,
            )
            nc.vector.tensor_reduce(
                out=s2[:, 1:2], in_=sumsq[:], axis=mybir.AxisListType.X,
                op=mybir.AluOpType.add,
            )
            # all-reduce the raw per-half sums (the half-fold happens after
            # the collective, off the pre-rendezvous critical path, via a
            # swapped-half readback + one add)
            cc_in = dram.tile([128, 2], F32)
            cc_out = dram.tile([128, 2], F32)
            nc.sync.dma_start(out=cc_in[:], in_=s2[:])
            nc.gpsimd.collective_compute(
                "AllReduce",
                mybir.AluOpType.add,
                replica_groups=[list(range(NCORES))],
                ins=[cc_in.opt()],
                outs=[cc_out.opt()],
            )
            raw = pp.tile([128, 2], F32)
            swp = pp.tile([128, 2], F32)
            nc.sync.dma_start(out=raw[:], in_=cc_out[:])
            nc.scalar.dma_start(out=swp[0:64, :], in_=cc_out[64:128, :])
            nc.scalar.dma_start(out=swp[64:128, :], in_=cc_out[0:64, :])
            stats_rd = pp.tile([128, 2], F32)
            nc.vector.tensor_tensor(
                out=stats_rd[:], in0=raw[:], in1=swp[:],
                op=mybir.AluOpType.add,
            )

            # scale = gamma/sqrt(var+eps), bias = beta - mean*scale.  All on
            # VectorE except the Sqrt (ScalarE-only) to minimize the serial
            # cross-engine semaphore hops on this dependency chain.
            mean = pp.tile([128, 8], F32)  # mean, msq, mean2, var, std, inv, -, m*s
            inv_n = 1.0 / float(N)
            nc.vector.tensor_scalar_mul(mean[:, 0:2], stats_rd[:, 0:2], inv_n)
            nc.vector.tensor_tensor(
                out=mean[:, 2:3], in0=mean[:, 0:1], in1=mean[:, 0:1],
                op=mybir.AluOpType.mult,
            )
            nc.vector.tensor_scalar(
                out=mean[:, 3:4], in0=mean[:, 1:2],
                scalar1=mean[:, 2:3], scalar2=EPS,
                op0=mybir.AluOpType.subtract, op1=mybir.AluOpType.add,
            )
            nc.scalar.activation(
                out=mean[:, 4:5], in_=mean[:, 3:4],
                func=mybir.ActivationFunctionType.Sqrt,
            )
            nc.vector.reciprocal(mean[:, 5:6], mean[:, 4:5])
            nc.vector.tensor_tensor(
                out=sb_full[:, 0:1], in0=mean[:, 5:6], in1=gb_sb[:, 0:1],
                op=mybir.AluOpType.mult,
            )
            nc.vector.tensor_tensor(
                out=mean[:, 7:8], in0=mean[:, 0:1], in1=sb_full[:, 0:1],
                op=mybir.AluOpType.mult,
            )
            nc.vector.tensor_tensor(
                out=sb_full[:, 1:2], in0=gb_sb[:, 1:2], in1=mean[:, 7:8],
                op=mybir.AluOpType.subtract,
            )

            # ---- Phase 2: normalize + ReLU (split ScalarE/VectorE), store ----
            with tc.tile_pool(name="norm", bufs=6) as nmp:
                ngrp = (NPAIR + P2G - 1) // P2G
                for g in range(ngrp):
                    prs = list(range(g * P2G, min(g * P2G + P2G, NPAIR)))
                    nm = nmp.tile([128, P2G * T], F16, tag="nm")
                    # normalize 4 pairs (2048 cols) per instruction, slices
                    # alternating between ScalarE and VectorE
                    nsl = (len(prs) + 3) // 4
                    for j2 in range(nsl):
                        w = min(4, len(prs) - 4 * j2) * T
                        dst = nm[:, j2 * 4 * T : j2 * 4 * T + w]
                        src = out_sb[:, prs[4 * j2] * T : prs[4 * j2] * T + w]
                        if j2 % 2 == 0:
                            nc.scalar.activation(
                                out=dst, in_=src,
                                func=mybir.ActivationFunctionType.Relu,
                                bias=sb_full[:, 1:2],
                                scale=sb_full[:, 0:1],
                            )
                        else:
                            nc.vector.tensor_scalar(
                                out=dst, in0=src,
                                scalar1=sb_full[:, 0:1],
                                scalar2=sb_full[:, 1:2],
                                op0=mybir.AluOpType.mult,
                                op1=mybir.AluOpType.add,
                            )
                            nc.vector.tensor_scalar_max(dst, dst, 0.0)
                    eng = nc.scalar if g % 2 == 0 else nc.sync
                    eng.dma_start(
                        out=y2_d[:, g * P2G * T : g * P2G * T + len(prs) * T],
                        in_=nm[:, 0 : len(prs) * T],
                    )
    return nc


_COMPILED = None


def _get_compiled():
    global _COMPILED
    if _COMPILED is None:
        nc = bacc.Bacc(
            "TRN2", target_bir_lowering=False, debug=False, num_devices=NCORES
        )
        _build(nc)
        nc.compile()
        _COMPILED = nc
    return _COMPILED


def _prep_core(x8u, nbr_idx, nbr_mask, c):
    """Build this core's streamed operand tensors ga/gc (e3m4 as uint8)."""
    sl = slice(c * NSH, (c + 1) * NSH)
    idx_c = nbr_idx[:, sl]
    msk_c = nbr_mask[:, sl]
    gat = x8u[idx_c]                                # [27, NSH, 32] uint8
    gat[~msk_c] = 0
    buf = np.zeros((NG * 4, NPAD, CIN), np.uint8)
    buf[:K, :NSH] = gat
    # [g, ti4, q, qt, v, c] -> [q, ti4, c, qt, g, v];  partition = ti4*32 + c
    G7 = np.ascontiguousarray(
        buf.reshape(NG, 4, NQ, QT, T, CIN).transpose(2, 1, 5, 3, 0, 4)
    ).reshape(NQ, 128, QT, NG, T)
    ga = np.ascontiguousarray(G7[:, :, :, 0:6, :]).reshape(NQ, 128, QT * 6 * T)
    gc = np.ascontiguousarray(G7[:, 0:96, :, 6, :]).reshape(NQ, 96, QT * T)
    return ga.view(F8NP), gc.view(F8NP)


def _prep_shared(weight, gamma, beta):
    wpad = np.zeros((NG * 4, CIN, COUT), np.float32)
    wpad[:K] = weight
    wb = wpad.astype(BF16NP).view(np.uint16)
    # [g, ti4, c, o] -> [ti4, c, g, o] -> [128, NG*COUT]
    w4 = np.ascontiguousarray(
        wb.reshape(NG, 4, CIN, COUT).transpose(1, 2, 0, 3)
    ).reshape(128, NG * COUT).view(BF16NP)
    gb = np.stack([gamma, beta], axis=1).astype(np.float32)  # [64, 2]
    gb = np.concatenate([gb, gb], axis=0)                    # [128, 2]
    return w4, gb


def run_on_hw(in_maps, **kwargs):
    nc = _get_compiled()
    return bass_utils.run_bass_kernel_spmd(
        nc, in_maps, core_ids=list(range(NCORES)), **kwargs
    )


def make_in_maps(x, weight, gamma, beta, nbr_idx, nbr_mask):
    x = np.asarray(x, np.float32)
    weight = np.asarray(weight, np.float32)
    nbr_idx = np.asarray(nbr_idx, np.int32)
    nbr_mask = np.asarray(nbr_mask)
    x8u = x.astype(F8NP).view(np.uint8)
    w4, gbv = _prep_shared(weight, np.asarray(gamma), np.asarray(beta))
    in_maps = []
    for c in range(NCORES):
        ga, gc = _prep_core(x8u, nbr_idx, nbr_mask, c)
        in_maps.append({"ga": ga, "gc": gc, "w4": w4, "gbeta": gbv})
    return in_maps


def unshard(results):
    """Per-core y2 [128, NPAIR*T] channel-major fp16 -> [N, COUT] f32."""
    outs = []
    for r in results:
        y2 = np.asarray(r["y2"]).astype(np.float32)
        y2 = y2.reshape(2, COUT, NPAIR, T)
        y = y2.transpose(2, 0, 3, 1).reshape(NPAD, COUT)
        outs.append(y[:NSH])
    return np.ascontiguousarray(np.concatenate(outs, axis=0))


def kernel(x, weight, gamma, beta, nbr_idx, nbr_mask):
    in_maps = make_in_maps(x, weight, gamma, beta, nbr_idx, nbr_mask)
    res = run_on_hw(in_maps)
    return unshard(res.results).astype(np.float32)


if __name__ == "__main__":
    rng = np.random.default_rng(0)
    x = rng.standard_normal((N, CIN), dtype=np.float32)
    w = (rng.standard_normal((K, CIN, COUT)) * 0.05).astype(np.float32)
    gamma = np.ones(COUT, np.float32)
    beta = np.zeros(COUT, np.float32)
    idx = rng.integers(0, N, (K, N)).astype(np.int32)
    msk = rng.integers(0, 2, (K, N)).astype(bool)
    y = kernel(x, w, gamma, beta, idx, msk)
    print("out", y.shape, y.dtype, float(np.abs(y).max()))
